# revision 14
# baseline (speedup 1.0000x reference)
"""Trainium2 Bass kernel for nn_CNN_69398081568992 (Go-board dense CNN).

Network (from the reference):
  x   = one-hot board planes from stone_idx [B,16,128] + color plane -> [B,17,19,19]
  r   = relu(BN(conv3x3(x, W0)+b0))                       # 17 -> 256
  3x residual blocks with SHARED weights Wm (256 -> 256, 3x3):
      d = relu(BN(conv(r)+bm)); r = relu(BN(conv(d)+bm) + r)
  out = relu(BN(conv1x1(r, We)+bE)).reshape(B, 361)
BatchNorm is training-mode: biased batch stats over (N,H,W) of the FULL batch.

Strategy: pure data parallel across 8 NeuronCores (32 images each), conv/BN
weights replicated.  Convs are computed as shift-matmuls over a zero-padded
21x21 board in SBUF (fp32r matmuls, N=19x20=380 windows to satisfy the
fp32r even-innermost ISA rule).  BN batch stats are made exact with one tiny
(2KB) AllReduce per BN layer.  The one-hot input build uses DVE/gpsimd
is_equal compares against an iota row plus a PE selector-matmul partition
reduce, thresholded with ACT Sign.
"""
import os
import sys

sys.path.insert(0, "/opt/trn_rl_repo")

import numpy as np

import concourse.bacc as bacc
import concourse.mybir as mybir
import concourse.tile as tile
import concourse.bass_utils as bass_utils
from contextlib import ExitStack

N_CORES = 8
B_TOT = 256
IMG = B_TOT // N_CORES      # 32 images per core
C = 256                     # channels
NBLOCK = 3
HW = 361                    # 19*19
NTOT = float(B_TOT * HW)    # BN normalizer (full batch!)
EPS = 1e-5

F32 = mybir.dt.float32
F32R = mybir.dt.float32r
BF16 = mybir.dt.bfloat16

# image block inside padded board buffers: 22 rows x 21 cols = 462 floats
BLK = 462


def _win(ap_owner, part_lo, part_hi, free_prefix, dy, dx):
    """Tap window [P, 19, 20] of a 462-elem padded image block.

    ap_owner[part_lo:part_hi, *free_prefix, base:base+399] reshaped to rows of
    21 then truncated to 20 cols.  Innermost count 20 is even (fp32r rule).
    """
    base = dy * 21 + dx
    ap = ap_owner[(slice(part_lo, part_hi), *free_prefix, slice(base, base + 399))]
    return ap.rearrange("p (r c) -> p r c", c=21)[:, :, 0:20]


def _interior(ap_owner, part_lo, part_hi, free_prefix):
    """[P, (s,) 19, 19] interior (row r, col c) -> flat (1+r)*21 + (1+c)."""
    ap = ap_owner[(slice(part_lo, part_hi), *free_prefix, slice(22, 421))]
    if len(ap.shape) == 3:  # extra slot dim kept (slice prefix)
        return ap.rearrange("p s (r c) -> p s r c", c=21)[:, :, :, 0:19]
    return ap.rearrange("p (r c) -> p r c", c=21)[:, :, 0:19]


def build_program(img=IMG, nblock=NBLOCK, n_cores=N_CORES, stop_after="full"):
    nc = bacc.Bacc("TRN2", target_bir_lowering=False, debug=False,
                   enable_asserts=True, num_devices=n_cores)
    S = img // 4  # image slots per partition group (g = i % 4, s = i // 4)
    ntot = float(img * n_cores * HW)  # BN batch-stats normalizer (global)

    idx_d = nc.dram_tensor("idx", [128, img * 16], F32, kind="ExternalInput").ap()
    iota_d = nc.dram_tensor("iota", [128, HW], F32, kind="ExternalInput").ap()
    sel_d = nc.dram_tensor("sel", [128, 256], BF16, kind="ExternalInput").ap()
    cpl_d = nc.dram_tensor("cpl", [4, S * HW], F32R, kind="ExternalInput").ap()
    w0_d = nc.dram_tensor("w0", [128, 9 * 256], F32R, kind="ExternalInput").ap()
    wm_d = nc.dram_tensor("wm", [128, 36 * 128], F32R, kind="ExternalInput").ap()
    we_d = nc.dram_tensor("we", [128, 2], F32R, kind="ExternalInput").ap()
    bnp_d = nc.dram_tensor("bnp", [128, 16], F32, kind="ExternalInput").ap()
    out_d = nc.dram_tensor("out", [img, HW], F32, kind="ExternalOutput").ap()

    rg = [list(range(n_cores))]

    with tile.TileContext(nc) as tc, ExitStack() as ctx:
        per = ctx.enter_context(tc.tile_pool(name="per", bufs=1))
        dram = ctx.enter_context(tc.tile_pool(name="dram", bufs=1, space="DRAM"))
        cps = ctx.enter_context(tc.tile_pool(name="cps", bufs=4, space="PSUM"))

        # ---- persistent tiles -------------------------------------------
        idx_sb = per.tile([128, img * 16], F32)
        iota_sb = per.tile([128, HW], F32)
        sel_sb = per.tile([128, 256], BF16)
        w0_sb = per.tile([128, 9, 256], F32R)
        wm_sb = per.tile([128, 36, 128], F32R)
        we_sb = per.tile([128, 2], F32R)
        bnp_sb = per.tile([128, 16], F32)
        r_c = per.tile([128, 2, img, 362], F32R)      # residual stream (compact)
        sums = per.tile([128, 2 * img], F32)
        sumsq = per.tile([128, 2 * img], F32)
        stats_sb = per.tile([128, 4], F32)
        gstats_sb = per.tile([128, 4], F32)
        a_sb = per.tile([128, 2], F32)
        b_sb = per.tile([128, 2], F32)
        m_t = per.tile([128, 2], F32)
        v_t = per.tile([128, 2], F32)
        msq_t = per.tile([128, 2], F32)
        rs_t = per.tile([128, 2], F32)
        sq_scr = per.tile([128, HW], F32)

        nc.sync.dma_start(idx_sb[:], idx_d)
        nc.sync.dma_start(iota_sb[:], iota_d)
        nc.sync.dma_start(sel_sb[:], sel_d)
        nc.sync.dma_start(w0_sb[:].rearrange("p a b -> p (a b)"), w0_d)
        nc.sync.dma_start(wm_sb[:].rearrange("p a b -> p (a b)"), wm_d)
        nc.sync.dma_start(we_sb[:], we_d)
        nc.sync.dma_start(bnp_sb[:], bnp_d)
        nc.gpsimd.memset(r_c[:].rearrange("p a b c -> p (a b c)").bitcast(mybir.dt.uint32), 0)

        # bnp columns: 0-1 b0 | 2-3 g0 | 4-5 be0 | 6-7 bm | 8-9 gm | 10-11 bem
        #              p0: 12 bE, 13 ge, 14 bee

        def bn_affine(gsum_ap, gsumsq_ap, g_ap, be_ap, a_ap, b_ap, mt, vt, mqt, rt):
            nc.vector.tensor_scalar_mul(mt, gsum_ap, 1.0 / ntot)
            nc.vector.tensor_scalar_mul(vt, gsumsq_ap, 1.0 / ntot)
            nc.vector.tensor_tensor(mqt, mt, mt, op=mybir.AluOpType.mult)
            nc.vector.tensor_tensor(vt, vt, mqt, op=mybir.AluOpType.subtract)
            nc.vector.tensor_scalar_add(vt, vt, EPS)
            nc.vector.reciprocal(vt, vt)
            nc.scalar.sqrt(rt, vt)                      # rsqrt(var+eps)
            nc.vector.tensor_tensor(a_ap, rt, g_ap, op=mybir.AluOpType.mult)
            nc.vector.tensor_tensor(mqt, mt, a_ap, op=mybir.AluOpType.mult)
            nc.vector.tensor_tensor(b_ap, be_ap, mqt, op=mybir.AluOpType.subtract)

        def bn_allreduce(layer):
            cin = dram.tile([128, 4], F32, name=f"ccin{layer}", tag=f"ccin{layer}")
            cout = dram.tile([128, 4], F32, name=f"ccout{layer}", tag=f"ccout{layer}",
                             addr_space="Shared")
            nc.sync.dma_start(cin[:], stats_sb[:])
            nc.gpsimd.collective_compute(
                "AllReduce", mybir.AluOpType.add, replica_groups=rg,
                ins=[cin[:]], outs=[cout[:]])
            nc.sync.dma_start(gstats_sb[:], cout[:])

        # ================= phase 1: one-hot input build ===================
        with tc.tile_pool(name="inp", bufs=1) as inp, \
             tc.tile_pool(name="cntp", bufs=3, space="PSUM") as cntp:
            x0 = inp.tile([128, S, BLK], F32R)
            e_tiles = [inp.tile([128, 16, HW], BF16, name=f"E{j}") for j in range(2)]
            nc.gpsimd.memset(x0[:].rearrange("p a b -> p (a b)").bitcast(mybir.dt.uint32), 0)
            # color plane: channel 16 of each group (per-slot DMAs keep APs <=3 dims)
            for g in range(4):
                for s in range(S):
                    dst = _interior(x0, 32 * g + 16, 32 * g + 17, (s,))
                    src = cpl_d[g:g + 1, s * HW:(s + 1) * HW].rearrange(
                        "p (r c) -> p r c", c=19)
                    nc.sync.dma_start(dst, src)

            for i in range(img):
                g, s = i % 4, i // 4
                E = e_tiles[i % 2]
                cnt = cntp.tile([128, HW], F32, name=f"cnt{i}", tag="cnt")
                for p in range(16):
                    eng = nc.vector if p < 8 else nc.gpsimd
                    eng.tensor_scalar(
                        E[:, p, :], iota_sb[:], idx_sb[:, i * 16 + p: i * 16 + p + 1],
                        None, op0=mybir.AluOpType.is_equal)
                for p in range(16):
                    off = 128 - (32 * g + p)
                    nc.tensor.matmul(
                        cnt[:, :],
                        sel_sb[:, off:off + 128], E[:, p, :],
                        start=(p == 0), stop=(p == 15))
                nc.scalar.activation(
                    _interior(x0, 32 * g, 32 * g + 16, (s,)),
                    cnt[32 * g:32 * g + 16, 0:361].rearrange("p (r c) -> p r c", c=19),
                    mybir.ActivationFunctionType.Sign)

            # ================= phase 2: conv0 (17 -> 256) =================
            for i in range(img):
                g, s = i % 4, i // 4
                for cob in range(2):
                    pt = cps.tile([128, 380], F32, name=f"c0p{i}_{cob}", tag="cp")
                    for tap in range(9):
                        dy, dx = divmod(tap, 3)
                        rhs = _win(x0, 32 * g, 32 * g + 17, (s,), dy, dx)
                        lhsT = w0_sb[32 * g:32 * g + 17, tap, cob * 128:(cob + 1) * 128]
                        nc.tensor.matmul(pt[:], lhsT, rhs,
                                         start=(tap == 0), stop=(tap == 8),
                                         tile_position=(32 * g, 0))
                    pv = pt[:, 0:380].rearrange("p (r c) -> p r c", c=20)[:, :, 0:19]
                    nc.scalar.activation(
                        r_c[:, cob, i, 0:361].rearrange("p (r c) -> p r c", c=19),
                        pv, mybir.ActivationFunctionType.Identity,
                        bias=bnp_sb[:, cob:cob + 1],
                        accum_out=sums[:, cob * img + i:cob * img + i + 1])
                    nc.scalar.activation(
                        sq_scr[:, 0:361], r_c[:, cob, i, 0:361],
                        mybir.ActivationFunctionType.Square,
                        accum_out=sumsq[:, cob * img + i:cob * img + i + 1])

        # BN0
        nc.vector.tensor_reduce(
            stats_sb[:, 0:2], sums[:].rearrange("p (b i) -> p b i", i=img),
            axis=mybir.AxisListType.X, op=mybir.AluOpType.add)
        nc.vector.tensor_reduce(
            stats_sb[:, 2:4], sumsq[:].rearrange("p (b i) -> p b i", i=img),
            axis=mybir.AxisListType.X, op=mybir.AluOpType.add)
        bn_allreduce(0)
        bn_affine(gstats_sb[:, 0:2], gstats_sb[:, 2:4],
                  bnp_sb[:, 2:4], bnp_sb[:, 4:6],
                  a_sb[:, 0:2], b_sb[:, 0:2], m_t[:], v_t[:], msq_t[:], rs_t[:])
        for cob in range(2):
            nc.scalar.activation(
                r_c[:, cob, :, :], r_c[:, cob, :, :],
                mybir.ActivationFunctionType.Relu,
                scale=a_sb[:, cob:cob + 1], bias=b_sb[:, cob:cob + 1])

        if stop_after == "bn0":
            nc.sync.dma_start(out_d, r_c[0:1, 0, :, 0:361].bitcast(F32))
            nblock = 0

        # ================= phase 3: residual blocks =======================
        with tc.tile_pool(name="blkp", bufs=1) as blkp:
            d_c = blkp.tile([128, 2, img, HW], BF16)
            stgs = [blkp.tile([128, 2, BLK], F32R, name=f"stg{j}") for j in range(2)]
            for st in stgs:
                nc.gpsimd.memset(st[:].rearrange("p a b -> p (a b)").bitcast(mybir.dt.uint32), 0)

            def mid_conv(src_getter, dst_tile, layer_tag):
                """3x3 conv (256->256) from per-image staged input; drains into
                dst_tile (bf16) with bias bm, accumulating sums/sumsq."""
                for i in range(img):
                    st = stgs[i % 2]
                    for blk in range(2):
                        nc.vector.tensor_copy(
                            _interior(st, 0, 128, (blk,)),
                            src_getter(blk, i).rearrange("p (r c) -> p r c", c=19))
                    for cob in range(2):
                        pt = cps.tile([128, 380], F32,
                                      name=f"{layer_tag}p{i}_{cob}", tag="cp")
                        first = True
                        for cib in range(2):
                            for tap in range(9):
                                dy, dx = divmod(tap, 3)
                                rhs = _win(st, 0, 128, (cib,), dy, dx)
                                t = (tap * 2 + cib) * 2 + cob
                                nc.tensor.matmul(
                                    pt[:], wm_sb[:, t, :], rhs,
                                    start=first, stop=(cib == 1 and tap == 8))
                                first = False
                        pv = pt[:, 0:380].rearrange("p (r c) -> p r c", c=20)[:, :, 0:19]
                        nc.scalar.activation(
                            dst_tile[:, cob, i, :].rearrange("p (r c) -> p r c", c=19),
                            pv, mybir.ActivationFunctionType.Identity,
                            bias=bnp_sb[:, 6 + cob:7 + cob],
                            accum_out=sums[:, cob * img + i:cob * img + i + 1])
                        nc.scalar.activation(
                            sq_scr[:, 0:361], dst_tile[:, cob, i, :],
                            mybir.ActivationFunctionType.Square,
                            accum_out=sumsq[:, cob * img + i:cob * img + i + 1])

            def stats_and_affine(layer):
                nc.vector.tensor_reduce(
                    stats_sb[:, 0:2], sums[:].rearrange("p (b i) -> p b i", i=img),
                    axis=mybir.AxisListType.X, op=mybir.AluOpType.add)
                nc.vector.tensor_reduce(
                    stats_sb[:, 2:4], sumsq[:].rearrange("p (b i) -> p b i", i=img),
                    axis=mybir.AxisListType.X, op=mybir.AluOpType.add)
                bn_allreduce(layer)
                bn_affine(gstats_sb[:, 0:2], gstats_sb[:, 2:4],
                          bnp_sb[:, 8:10], bnp_sb[:, 10:12],
                          a_sb[:, 0:2], b_sb[:, 0:2],
                          m_t[:], v_t[:], msq_t[:], rs_t[:])

            for b in range(nblock):
                # conv A: r -> d, then d = relu(bn(d))
                mid_conv(lambda blk, i: r_c[:, blk, i, 0:361], d_c, f"b{b}a")
                stats_and_affine(1 + 2 * b)
                for cob in range(2):
                    nc.scalar.activation(
                        d_c[:, cob, :, :], d_c[:, cob, :, :],
                        mybir.ActivationFunctionType.Relu,
                        scale=a_sb[:, cob:cob + 1], bias=b_sb[:, cob:cob + 1])
                # conv B: d -> y2 (overwrites d), then r = relu(a*y2 + b + r)
                mid_conv(lambda blk, i: d_c[:, blk, i, :], d_c, f"b{b}b")
                stats_and_affine(2 + 2 * b)
                for cob in range(2):
                    nc.vector.scalar_tensor_tensor(
                        r_c[:, cob, :, 0:361], d_c[:, cob, :, :],
                        a_sb[:, cob:cob + 1], r_c[:, cob, :, 0:361],
                        op0=mybir.AluOpType.mult, op1=mybir.AluOpType.add)
                    nc.vector.tensor_scalar(
                        r_c[:, cob, :, 0:361], r_c[:, cob, :, 0:361],
                        b_sb[:, cob:cob + 1], 0.0,
                        op0=mybir.AluOpType.add, op1=mybir.AluOpType.max)
                if stop_after == f"block{b + 1}":
                    nc.sync.dma_start(out_d, r_c[0:1, 0, :, 0:361].bitcast(F32))
                    break

        # ================= phase 4: final 1x1 conv + BN ===================
        if stop_after != "full":
            pass
        else:
          with tc.tile_pool(name="finp", bufs=1) as finp, \
             tc.tile_pool(name="fps", bufs=3, space="PSUM") as fps:
            oy = finp.tile([1, img, 362], F32)
            oy_sums = finp.tile([1, img], F32)
            oy_sumsq = finp.tile([1, img], F32)
            for i in range(img):
                pt = fps.tile([1, 362], F32, name=f"fp{i}", tag="fp")
                for ch in range(2):
                    nc.tensor.matmul(pt[:], we_sb[:, ch:ch + 1], r_c[:, ch, i, :],
                                     start=(ch == 0), stop=(ch == 1))
                nc.scalar.activation(
                    oy[0:1, i, 0:361], pt[0:1, 0:361],
                    mybir.ActivationFunctionType.Identity,
                    bias=bnp_sb[0:1, 12:13],
                    accum_out=oy_sums[0:1, i:i + 1])
                nc.scalar.activation(
                    sq_scr[0:1, 0:361], oy[0:1, i, 0:361],
                    mybir.ActivationFunctionType.Square,
                    accum_out=oy_sumsq[0:1, i:i + 1])

            nc.vector.memset(stats_sb[:], 0.0)
            nc.vector.tensor_reduce(stats_sb[0:1, 0:1], oy_sums[0:1, :],
                                    axis=mybir.AxisListType.X, op=mybir.AluOpType.add)
            nc.vector.tensor_reduce(stats_sb[0:1, 1:2], oy_sumsq[0:1, :],
                                    axis=mybir.AxisListType.X, op=mybir.AluOpType.add)
            bn_allreduce(7)
            bn_affine(gstats_sb[0:1, 0:1], gstats_sb[0:1, 1:2],
                      bnp_sb[0:1, 13:14], bnp_sb[0:1, 14:15],
                      a_sb[0:1, 0:1], b_sb[0:1, 0:1],
                      m_t[0:1, 0:1], v_t[0:1, 0:1], msq_t[0:1, 0:1], rs_t[0:1, 0:1])
            nc.scalar.activation(
                oy[0:1, :, 0:361], oy[0:1, :, 0:361],
                mybir.ActivationFunctionType.Relu,
                scale=a_sb[0:1, 0:1], bias=b_sb[0:1, 0:1])
            nc.sync.dma_start(out_d, oy[0:1, :, 0:361])

    nc.compile()
    return nc


def prepare_inputs(stone_idx, color, W0, b0, g0, be0, Wm, bm, gm, bem,
                   We, bE, ge, bee, img=IMG, n_cores=N_CORES):
    """Host-side sharding + layout prep.  Returns in_maps (one dict/core)."""
    import ml_dtypes
    bf16 = ml_dtypes.bfloat16
    S = img // 4

    stone_idx = np.asarray(stone_idx)
    color = np.asarray(color, dtype=np.float32)
    W0 = np.asarray(W0, dtype=np.float32)
    Wm = np.asarray(Wm, dtype=np.float32)
    We = np.asarray(We, dtype=np.float32)

    iota_np = np.tile(np.arange(HW, dtype=np.float32), (128, 1))
    # sliding one-hot-column selector: sel[:, 128] == 1, so the [128,128]
    # slice starting at col 128-k has exactly column k set.
    sel_np = np.zeros((128, 256), dtype=np.float32)
    sel_np[:, 128] = 1.0
    sel_np = sel_np.astype(bf16)

    w0_np = np.zeros((128, 9, 256), dtype=np.float32)
    for g in range(4):
        for tap in range(9):
            dy, dx = divmod(tap, 3)
            w0_np[32 * g:32 * g + 17, tap, :] = W0[:, :, dy, dx].T
    w0_np = w0_np.reshape(128, 9 * 256)

    wm_np = np.zeros((128, 36, 128), dtype=np.float32)
    for tap in range(9):
        dy, dx = divmod(tap, 3)
        for cib in range(2):
            for cob in range(2):
                t = (tap * 2 + cib) * 2 + cob
                wm_np[:, t, :] = Wm[cob * 128:(cob + 1) * 128,
                                    cib * 128:(cib + 1) * 128, dy, dx].T
    wm_np = wm_np.reshape(128, 36 * 128)

    we_np = np.zeros((128, 2), dtype=np.float32)
    we_np[:, 0] = We[0, 0:128, 0, 0]
    we_np[:, 1] = We[0, 128:256, 0, 0]

    bnp_np = np.zeros((128, 16), dtype=np.float32)
    for cob in range(2):
        sl = slice(cob * 128, (cob + 1) * 128)
        bnp_np[:, 0 + cob] = b0[sl]
        bnp_np[:, 2 + cob] = g0[sl]
        bnp_np[:, 4 + cob] = be0[sl]
        bnp_np[:, 6 + cob] = bm[sl]
        bnp_np[:, 8 + cob] = gm[sl]
        bnp_np[:, 10 + cob] = bem[sl]
    bnp_np[0, 12] = float(np.asarray(bE).reshape(-1)[0])
    bnp_np[0, 13] = float(np.asarray(ge).reshape(-1)[0])
    bnp_np[0, 14] = float(np.asarray(bee).reshape(-1)[0])

    in_maps = []
    for c in range(n_cores):
        sl = slice(c * img, (c + 1) * img)
        idx_core = stone_idx[sl].astype(np.float32)          # [img,16,128]
        idx_np = idx_core.transpose(2, 0, 1).reshape(128, img * 16).copy()
        col_core = color[sl]
        cpl_np = np.zeros((4, S * HW), dtype=np.float32)
        for i in range(img):
            g, s = i % 4, i // 4
            cpl_np[g, s * HW:(s + 1) * HW] = col_core[i]
        in_maps.append({
            "idx": idx_np, "iota": iota_np, "sel": sel_np, "cpl": cpl_np,
            "w0": w0_np, "wm": wm_np, "we": we_np, "bnp": bnp_np,
        })
    return in_maps


_NC_CACHE = {}


def kernel(**inputs) -> np.ndarray:
    img, n_cores = IMG, N_CORES
    key = (img, NBLOCK, n_cores)
    if key not in _NC_CACHE:
        _NC_CACHE[key] = build_program(img, NBLOCK, n_cores)
    nc = _NC_CACHE[key]
    in_maps = prepare_inputs(**inputs, img=img, n_cores=n_cores)
    res = bass_utils.run_bass_kernel_spmd(nc, in_maps, core_ids=list(range(n_cores)))
    return np.concatenate([res.results[c]["out"] for c in range(n_cores)], axis=0)
